# revision 6
# baseline (speedup 1.0000x reference)
"""HB-LSTM cell fused Trainium2 kernel, data-parallel over 8 NeuronCores.

Computes, for gate order (f, i, o, u, k):
    pre  = x @ Wx[g].T + bx[g] + h_prev @ Uh[g].T + bh[g]
    f,i,o,u = sigmoid(pre[0..3]);  c = tanh(pre[4])
    kp = u*c + (1-u)*kp_prev
    k  = f*k_prev + i*kp
    h  = o*tanh(k)
Returns (h, k, kp), each [B, H] float32.

Sharding: batch dim B=65536 split across 8 cores (8192 rows each); weight
stacks replicated to every core.

Per-core structure (64 b-tiles of 128 rows, groups of GROUP=8):
  - All casting DMAs (x/h/k/kp loads fp32->bf16, output stores bf16->fp32)
    on the Pool SWDGE ring; stores for group g issue after loads of g+1 so
    Pool never stalls the load pipeline.
  - x/h transposed to feature-major on the PE (identity matmul, bf16) into
    1-bank PSUM tiles; DVE 2x-mode copies them to SBUF lhsT tiles.
  - 5-gate pre-activations accumulate in one [128,1280] PSUM tile per
    b-tile: 3 K=1 ones-matmuls broadcast the full bias row, then 12 bf16
    matmuls accumulate both GEMMs.
  - ACT: one wide sigmoid [128,1024] + tanh [128,256] per b-tile, bf16 out.
  - Elementwise tail entirely in bf16 on DVE (2x mode), batched per group.
"""

import contextlib

import numpy as np

import concourse.bacc as bacc
import concourse.mybir as mybir
from concourse import tile
from concourse.bass_utils import run_bass_kernel_spmd

N_CORES = 8
B = 65536
IN = 256
H = 256
G5 = 5
BL = B // N_CORES          # rows per core
NT = BL // 128             # 64 b-tiles per core
GROUP = 8                  # b-tiles per DMA group
NG = NT // GROUP
DG = G5 * H                # 1280 = all-gate column span
F32 = mybir.dt.float32
BF16 = mybir.dt.bfloat16
AF = mybir.ActivationFunctionType

# Bench mode: when set, the main loop runs LOOP_N times inside a hardware
# For_i loop so device time dominates RPC overhead in wall-clock.
LOOP_N = None

_CACHE = {}


def _build():
    if "nc" in _CACHE:
        return _CACHE["nc"]

    nc = bacc.Bacc("TRN2", target_bir_lowering=False, debug=False,
                   num_devices=N_CORES)

    x_d = nc.dram_tensor("x", [BL, IN], F32, kind="ExternalInput")
    h_d = nc.dram_tensor("h_prev", [BL, H], F32, kind="ExternalInput")
    k_d = nc.dram_tensor("k_prev", [BL, H], F32, kind="ExternalInput")
    kp_d = nc.dram_tensor("kp_prev", [BL, H], F32, kind="ExternalInput")
    wx_d = nc.dram_tensor("Wx", [G5, H, IN], F32, kind="ExternalInput")
    bx_d = nc.dram_tensor("bx", [G5, H], F32, kind="ExternalInput")
    uh_d = nc.dram_tensor("Uh", [G5, H, H], F32, kind="ExternalInput")
    bh_d = nc.dram_tensor("bh", [G5, H], F32, kind="ExternalInput")
    ho_d = nc.dram_tensor("h_out", [BL, H], F32, kind="ExternalOutput")
    ko_d = nc.dram_tensor("k_out", [BL, H], F32, kind="ExternalOutput")
    kpo_d = nc.dram_tensor("kp_out", [BL, H], F32, kind="ExternalOutput")

    with tile.TileContext(nc) as tc:
        with tc.tile_pool(name="const", bufs=1) as cpool:
            # WT[(side, c)]: [128 (i-chunk c), 1280 (g,h)] bf16 = matmul rhs
            WT = {}
            for side in ("x", "h"):
                for c in range(2):
                    WT[side, c] = cpool.tile([128, DG], BF16,
                                             name=f"WT_{side}{c}",
                                             tag=f"WT_{side}{c}")
            bs16 = cpool.tile([1, DG], BF16, tag="bs16")
            ones16 = cpool.tile([1, 128], BF16, tag="ones16")
            ident = cpool.tile([128, 128], BF16, tag="ident")

            with tc.tile_pool(name="binit", bufs=1) as bpool:
                # identity matrix for PE transposes, built first so the Pool
                # engine frees up for the group-0 activation loads
                onesq = bpool.tile([128, 128], BF16, tag="onesq")
                nc.vector.memset(onesq[:], 1.0)
                nc.gpsimd.affine_select(
                    ident[:], onesq[:], pattern=[[-1, 128]], base=0,
                    channel_multiplier=1,
                    compare_op=mybir.AluOpType.is_equal, fill=0.0)
                nc.vector.memset(ones16[:], 1.0)

                # --- weights: fp32 HWDGE loads (SP), engine casts to bf16,
                # then feature-major xbar transposes split SP/ACT ---
                with tc.tile_pool(name="wload", bufs=1) as wload:
                    w32 = {}
                    w16 = {}
                    for side, w_d in (("x", wx_d), ("h", uh_d)):
                        w32[side] = wload.tile([128, 2 * G5, IN], F32,
                                               name=f"w32{side}",
                                               tag=f"w32{side}")
                        wsrc = w_d.ap().rearrange(
                            "g (hc p) i -> p (g hc) i", p=128)
                        nc.sync.dma_start(w32[side][:], wsrc)
                        w16[side] = wload.tile([128, 2 * G5, IN], BF16,
                                               name=f"w16{side}",
                                               tag=f"w16{side}")
                    nc.vector.tensor_copy(w16["x"][:], w32["x"][:])
                    nc.scalar.copy(w16["h"][:], w32["h"][:])
                    for side in ("x", "h"):
                        eng = nc.sync if side == "x" else nc.scalar
                        for g in range(G5):
                            for hc in range(2):
                                col = g * H + hc * 128
                                for c in range(2):
                                    eng.dma_start(
                                        WT[side, c][:, col:col + 128],
                                        w16[side][:, g * 2 + hc,
                                                  c * 128:(c + 1) * 128],
                                        transpose=True)

                # --- bias row bs16 [1,1280] bf16 (K=1 PE bias matmuls) ---
                bxr = bpool.tile([G5, H], F32, tag="bxr")
                nc.scalar.dma_start(bxr[:], bx_d.ap())
                bhr = bpool.tile([G5, H], F32, tag="bhr")
                nc.scalar.dma_start(bhr[:], bh_d.ap())
                bsr = bpool.tile([G5, H], F32, tag="bsr")
                nc.vector.tensor_add(bsr[:], bxr[:], bhr[:])
                bsg = bpool.tile([G5, H], BF16, tag="bsg")
                nc.vector.tensor_copy(bsg[:], bsr[:])
                # flatten [5,256] -> one row [1,1280] (partition-major)
                nc.scalar.dma_start(bs16[:], bsg[:])

            # --- main loop ---
            x_t = x_d.ap().rearrange("(n p) i -> p n i", p=128)
            h_t = h_d.ap().rearrange("(n p) i -> p n i", p=128)
            k_t = k_d.ap().rearrange("(n p) i -> p n i", p=128)
            kp_t = kp_d.ap().rearrange("(n p) i -> p n i", p=128)
            ho_t = ho_d.ap().rearrange("(n p) i -> p n i", p=128)
            ko_t = ko_d.ap().rearrange("(n p) i -> p n i", p=128)
            kpo_t = kpo_d.ap().rearrange("(n p) i -> p n i", p=128)

            loop_cm = (tc.For_i(0, LOOP_N, 1) if LOOP_N
                       else contextlib.nullcontext())
            with tc.tile_pool(name="io", bufs=2) as io, \
                 tc.tile_pool(name="work", bufs=2) as work, \
                 tc.tile_pool(name="tmp", bufs=1) as tmp, \
                 tc.tile_pool(name="psum", bufs=2, space="PSUM") as pp, \
                 tc.tile_pool(name="psumt", bufs=2, space="PSUM") as ppx, \
                 loop_cm:
                prev = None
                for gi in range(NG):
                    nsl = slice(gi * GROUP, (gi + 1) * GROUP)
                    # Pool SWDGE cast loads (fp32 -> bf16 in flight)
                    x16 = io.tile([128, GROUP, IN], BF16, tag="x16")
                    nc.gpsimd.dma_start(x16[:], x_t[:, nsl, :])
                    h16 = io.tile([128, GROUP, H], BF16, tag="h16")
                    nc.gpsimd.dma_start(h16[:], h_t[:, nsl, :])
                    kpr16 = io.tile([128, GROUP, H], BF16, tag="kpr16")
                    nc.gpsimd.dma_start(kpr16[:], k_t[:, nsl, :])
                    kppr16 = io.tile([128, GROUP, H], BF16, tag="kppr16")
                    nc.gpsimd.dma_start(kppr16[:], kp_t[:, nsl, :])
                    # previous group's stores (fp32, HWDGE on the idle SP ring)
                    if prev is not None:
                        pkp, pk, ph, psl = prev
                        nc.sync.dma_start(kpo_t[:, psl, :], pkp[:])
                        nc.sync.dma_start(ko_t[:, psl, :], pk[:])
                        nc.sync.dma_start(ho_t[:, psl, :], ph[:])

                    # PE transposes -> 1-bank PSUM tiles -> DVE 2x copies
                    xT = work.tile([128, 2, GROUP, 128], BF16, tag="xT")
                    hT = work.tile([128, 2, GROUP, 128], BF16, tag="hT")
                    for a16, aT in ((x16, xT), (h16, hT)):
                        for c in range(2):
                            pt = ppx.tile([128, GROUP, 128], BF16, tag="pt")
                            for j in range(GROUP):
                                nc.tensor.transpose(
                                    pt[:, j, :],
                                    a16[:, j, c * 128:(c + 1) * 128],
                                    ident[:])
                            nc.vector.tensor_copy(aT[:, c], pt[:])

                    gates = work.tile([128, GROUP, 1024], BF16, tag="gates")
                    cg = work.tile([128, GROUP, 256], BF16, tag="cg")
                    for j in range(GROUP):
                        ps = pp.tile([128, DG], F32, tag="ps")
                        # bias broadcast: K=1 ones-matmuls, one per bank
                        for n0 in range(0, DG, 512):
                            n1 = min(n0 + 512, DG)
                            nc.tensor.matmul(ps[:, n0:n1], ones16[:],
                                             bs16[:, n0:n1],
                                             start=True, stop=False)
                        for si, (side, aT) in enumerate((("x", xT),
                                                         ("h", hT))):
                            for c in range(2):
                                lhsT = aT[:, c, j, :]
                                last = si == 1 and c == 1
                                for n0 in range(0, DG, 512):
                                    n1 = min(n0 + 512, DG)
                                    nc.tensor.matmul(
                                        ps[:, n0:n1], lhsT,
                                        WT[side, c][:, n0:n1],
                                        start=False, stop=last)
                        nc.scalar.activation(gates[:, j, :], ps[:, 0:1024],
                                             AF.Sigmoid)
                        nc.scalar.activation(cg[:, j, :], ps[:, 1024:DG],
                                             AF.Tanh)

                    # bf16 elementwise tail, batched over the group (DVE 2x)
                    f_ = gates[:, :, 0:256]
                    i_ = gates[:, :, 256:512]
                    o_ = gates[:, :, 512:768]
                    u_ = gates[:, :, 768:1024]
                    kp_o = io.tile([128, GROUP, H], F32, tag="kp_o")
                    k_o = io.tile([128, GROUP, H], F32, tag="k_o")
                    h_o = io.tile([128, GROUP, H], F32, tag="h_o")
                    d = tmp.tile([128, GROUP, H], BF16, tag="d")
                    nc.vector.tensor_sub(d[:], cg[:], kppr16[:])
                    e = tmp.tile([128, GROUP, H], BF16, tag="e")
                    nc.vector.tensor_mul(e[:], u_, d[:])
                    m = tmp.tile([128, GROUP, H], BF16, tag="m")
                    nc.vector.tensor_mul(m[:], f_, kpr16[:])
                    # kp in bf16 (2x) for the k recurrence, fp32 for the store
                    kp16 = tmp.tile([128, GROUP, H], BF16, tag="kp16")
                    nc.vector.tensor_add(kp16[:], e[:], kppr16[:])
                    nc.vector.tensor_add(kp_o[:], e[:], kppr16[:])
                    n2 = tmp.tile([128, GROUP, H], BF16, tag="n2")
                    nc.vector.tensor_mul(n2[:], i_, kp16[:])
                    nc.vector.tensor_add(k_o[:], m[:], n2[:])
                    tk = tmp.tile([128, GROUP, H], BF16, tag="tk")
                    nc.scalar.activation(tk[:], k_o[:], AF.Tanh)
                    nc.vector.tensor_mul(h_o[:], o_, tk[:])

                    prev = (kp_o, k_o, h_o, nsl)

                pkp, pk, ph, psl = prev
                nc.sync.dma_start(kpo_t[:, psl, :], pkp[:])
                nc.sync.dma_start(ko_t[:, psl, :], pk[:])
                nc.sync.dma_start(ho_t[:, psl, :], ph[:])

    nc.compile()
    _CACHE["nc"] = nc
    return nc


def kernel(x, h_prev, k_prev, kp_prev, Wx, bx, Uh, bh):
    x = np.asarray(x, dtype=np.float32)
    h_prev = np.asarray(h_prev, dtype=np.float32)
    k_prev = np.asarray(k_prev, dtype=np.float32)
    kp_prev = np.asarray(kp_prev, dtype=np.float32)
    Wx = np.ascontiguousarray(np.asarray(Wx, dtype=np.float32))
    bx = np.ascontiguousarray(np.asarray(bx, dtype=np.float32))
    Uh = np.ascontiguousarray(np.asarray(Uh, dtype=np.float32))
    bh = np.ascontiguousarray(np.asarray(bh, dtype=np.float32))

    nc = _build()
    in_maps = []
    for c in range(N_CORES):
        sl = slice(c * BL, (c + 1) * BL)
        in_maps.append({
            "x": np.ascontiguousarray(x[sl]),
            "h_prev": np.ascontiguousarray(h_prev[sl]),
            "k_prev": np.ascontiguousarray(k_prev[sl]),
            "kp_prev": np.ascontiguousarray(kp_prev[sl]),
            "Wx": Wx, "bx": bx, "Uh": Uh, "bh": bh,
        })
    res = run_bass_kernel_spmd(nc, in_maps, list(range(N_CORES)))
    h_out = np.concatenate([res.results[c]["h_out"] for c in range(N_CORES)],
                           axis=0)
    k_out = np.concatenate([res.results[c]["k_out"] for c in range(N_CORES)],
                           axis=0)
    kp_out = np.concatenate([res.results[c]["kp_out"]
                             for c in range(N_CORES)], axis=0)
    return (h_out, k_out, kp_out)


# revision 22
# speedup vs baseline: 1.2186x; 1.2186x over previous
"""HB-LSTM cell fused Trainium2 kernel, data-parallel over 8 NeuronCores.

Computes, for gate order (f, i, o, u, k):
    pre  = x @ Wx[g].T + bx[g] + h_prev @ Uh[g].T + bh[g]
    f,i,o,u = sigmoid(pre[0..3]);  c = tanh(pre[4])
    kp = u*c + (1-u)*kp_prev
    k  = f*k_prev + i*kp
    h  = o*tanh(k)
Returns (h, k, kp), each [B, H] float32.

Sharding: batch dim B=65536 split across 8 cores (8192 rows each); weight
stacks replicated to every core.

Per-core structure (64 b-tiles of 128 rows, groups of GROUP=8):
  - fp16 on-chip compute throughout: same PE/DVE throughput as bf16 but 8x
    the mantissa, so the fused tail stays well inside the error budget.
  - All casting DMAs (x/h/k/kp loads fp32->fp16, output stores fp16->fp32)
    on the Pool SWDGE ring; stores for group g issue after loads of g+1 so
    Pool never stalls the load pipeline.
  - x/h transposed to feature-major on the PE (identity matmul, fp16) into
    1-bank PSUM tiles; DVE 2x-mode copies them to SBUF lhsT tiles.
  - 5-gate pre-activations accumulate in one [128,1280] PSUM tile per
    b-tile; bias pre-fills the tile (K=1 ones-matmul on PE for cols 0:512,
    DVE broadcast copy for 512:1280), then 12 fp16 matmuls accumulate both
    GEMMs on top.
  - ACT: one wide sigmoid [128,1024] + tanh [128,256] per b-tile, fp16 out.
  - Elementwise tail in fp16 on DVE (2x mode), batched per half-group.
"""

import contextlib

import numpy as np

import concourse.bacc as bacc
import concourse.mybir as mybir
from concourse import tile
from concourse.bass_utils import run_bass_kernel_spmd

N_CORES = 8
B = 65536
IN = 256
H = 256
G5 = 5
BL = B // N_CORES          # rows per core
NT = BL // 128             # 64 b-tiles per core
GROUP = 8                  # b-tiles per DMA group
NG = NT // GROUP
DG = G5 * H                # 1280 = all-gate column span
F32 = mybir.dt.float32
F16 = mybir.dt.float16
F8 = mybir.dt.float8e4
AF = mybir.ActivationFunctionType
DR = mybir.MatmulPerfMode.DoubleRow

# Bench mode: when set, the main loop runs LOOP_N times inside a hardware
# For_i loop so device time dominates RPC overhead in wall-clock.
LOOP_N = None

_CACHE = {}


def _build():
    if "nc" in _CACHE:
        return _CACHE["nc"]

    nc = bacc.Bacc("TRN2", target_bir_lowering=False, debug=False,
                   num_devices=N_CORES)

    x_d = nc.dram_tensor("x", [BL, IN], F32, kind="ExternalInput")
    h_d = nc.dram_tensor("h_prev", [BL, H], F32, kind="ExternalInput")
    k_d = nc.dram_tensor("k_prev", [BL, H], F32, kind="ExternalInput")
    kp_d = nc.dram_tensor("kp_prev", [BL, H], F32, kind="ExternalInput")
    wx_d = nc.dram_tensor("Wx", [G5, H, IN], F32, kind="ExternalInput")
    bx_d = nc.dram_tensor("bx", [G5, H], F32, kind="ExternalInput")
    uh_d = nc.dram_tensor("Uh", [G5, H, H], F32, kind="ExternalInput")
    bh_d = nc.dram_tensor("bh", [G5, H], F32, kind="ExternalInput")
    ho_d = nc.dram_tensor("h_out", [BL, H], F32, kind="ExternalOutput")
    ko_d = nc.dram_tensor("k_out", [BL, H], F32, kind="ExternalOutput")
    kpo_d = nc.dram_tensor("kp_out", [BL, H], F32, kind="ExternalOutput")

    with tile.TileContext(nc) as tc:
        with tc.tile_pool(name="const", bufs=1) as cpool:
            # WT[(side, c)]: [128 (i-chunk c), 1280 (g,h)] bf16 = matmul rhs
            WT = {}
            for side in ("x", "h"):
                for c in range(2):
                    WT[side, c] = cpool.tile([128, DG], F16,
                                             name=f"WT_{side}{c}",
                                             tag=f"WT_{side}{c}")
            # fp8 DoubleRow bias operands: bias = 0.5*r0 + 0.5*r1 where
            # r0 = fp8(2b), r1 = fp8(2b - r0) (residual encoding, err ~2e-4)
            ones8 = cpool.tile([1, 2, 128], F8, tag="ones8")
            b8 = cpool.tile([1, 2, DG], F8, tag="b8")
            ident = cpool.tile([128, 128], F16, tag="ident")

            with tc.tile_pool(name="binit", bufs=1) as bpool:
                # identity matrix for PE transposes, built first so the Pool
                # engine frees up for the group-0 activation loads
                onesq = bpool.tile([128, 128], F16, tag="onesq")
                nc.vector.memset(onesq[:], 1.0)
                nc.gpsimd.affine_select(
                    ident[:], onesq[:], pattern=[[-1, 128]], base=0,
                    channel_multiplier=1,
                    compare_op=mybir.AluOpType.is_equal, fill=0.0)
                nc.vector.memset(ones8[:], 0.5)

                # fp32 identity for the weight PE transposes
                ident32 = bpool.tile([128, 128], F32, tag="ident32")
                onesq32 = bpool.tile([128, 128], F32, tag="onesq32")
                nc.vector.memset(onesq32[:], 1.0)
                nc.gpsimd.affine_select(
                    ident32[:], onesq32[:], pattern=[[-1, 128]], base=0,
                    channel_multiplier=1,
                    compare_op=mybir.AluOpType.is_equal, fill=0.0)

                # --- weights: fp32 HWDGE loads (SP/ACT in parallel), PE
                # transposes to feature-major, DVE copies cast to fp16 ---
                with tc.tile_pool(name="wload", bufs=1) as wload, \
                     tc.tile_pool(name="wps", bufs=2, space="PSUM") as wps:
                    w32 = {}
                    for side, w_d, eng in (("x", wx_d, nc.sync),
                                           ("h", uh_d, nc.scalar)):
                        w32[side] = wload.tile([128, 2 * G5, IN], F32,
                                               name=f"w32{side}",
                                               tag=f"w32{side}")
                        wsrc = w_d.ap().rearrange(
                            "g (hc p) i -> p (g hc) i", p=128)
                        eng.dma_start(w32[side][:], wsrc)
                    for side in ("x", "h"):
                        for c in range(2):
                            for gh0 in (0, 5):
                                pt32 = wps.tile([128, 5, 128], F32,
                                                tag="pt32")
                                for t in range(5):
                                    nc.tensor.transpose(
                                        pt32[:, t, :],
                                        w32[side][:, gh0 + t,
                                                  c * 128:(c + 1) * 128],
                                        ident32[:])
                                nc.scalar.copy(
                                    WT[side, c][:, gh0 * 128:
                                                (gh0 + 5) * 128],
                                    pt32[:])

                # --- bias rows (fp8 residual pair for DoubleRow matmul) ---
                bxr = bpool.tile([G5, H], F32, tag="bxr")
                nc.scalar.dma_start(bxr[:], bx_d.ap())
                bhr = bpool.tile([G5, H], F32, tag="bhr")
                nc.scalar.dma_start(bhr[:], bh_d.ap())
                bsr = bpool.tile([G5, H], F32, tag="bsr")
                nc.vector.tensor_add(bsr[:], bxr[:], bhr[:])
                bsd = bpool.tile([G5, H], F32, tag="bsd")
                nc.vector.tensor_scalar_mul(bsd[:], bsr[:], 2.0)
                r0 = bpool.tile([G5, H], F8, tag="r0")
                nc.vector.tensor_copy(r0[:], bsd[:])
                res = bpool.tile([G5, H], F32, tag="res")
                nc.vector.tensor_sub(res[:], bsd[:], r0[:])
                r1 = bpool.tile([G5, H], F8, tag="r1")
                nc.vector.tensor_copy(r1[:], res[:])
                # flatten [5,256] -> one row [1,1280] (partition-major)
                nc.scalar.dma_start(b8[:, 0, :], r0[:])
                nc.scalar.dma_start(b8[:, 1, :], r1[:])

            # --- main loop ---
            x_t = x_d.ap().rearrange("(n p) i -> p n i", p=128)
            h_t = h_d.ap().rearrange("(n p) i -> p n i", p=128)
            k_t = k_d.ap().rearrange("(n p) i -> p n i", p=128)
            kp_t = kp_d.ap().rearrange("(n p) i -> p n i", p=128)
            ho_t = ho_d.ap().rearrange("(n p) i -> p n i", p=128)
            ko_t = ko_d.ap().rearrange("(n p) i -> p n i", p=128)
            kpo_t = kpo_d.ap().rearrange("(n p) i -> p n i", p=128)

            loop_cm = (tc.For_i(0, LOOP_N, 1) if LOOP_N
                       else contextlib.nullcontext())
            with tc.tile_pool(name="io", bufs=2) as io, \
                 tc.tile_pool(name="work", bufs=2) as work, \
                 tc.tile_pool(name="tmp", bufs=1) as tmp, \
                 tc.tile_pool(name="psum", bufs=2, space="PSUM") as pp, \
                 tc.tile_pool(name="psumt", bufs=2, space="PSUM") as ppx, \
                 loop_cm:
                prev = None
                for gi in range(NG):
                    nsl = slice(gi * GROUP, (gi + 1) * GROUP)
                    # Pool SWDGE cast loads (fp32 -> bf16 in flight)
                    x16 = io.tile([128, GROUP, IN], F16, tag="x16")
                    nc.gpsimd.dma_start(x16[:], x_t[:, nsl, :])
                    h16 = io.tile([128, GROUP, H], F16, tag="h16")
                    nc.gpsimd.dma_start(h16[:], h_t[:, nsl, :])
                    kpr16 = io.tile([128, GROUP, H], F16, tag="kpr16")
                    nc.gpsimd.dma_start(kpr16[:], k_t[:, nsl, :])
                    kppr16 = io.tile([128, GROUP, H], F16, tag="kppr16")
                    nc.gpsimd.dma_start(kppr16[:], kp_t[:, nsl, :])
                    # previous group's stores (fp16->fp32 cast on Pool SWDGE)
                    if prev is not None:
                        pkp, pk, ph, psl = prev
                        nc.gpsimd.dma_start(kpo_t[:, psl, :], pkp[:])
                        nc.gpsimd.dma_start(ko_t[:, psl, :], pk[:])
                        nc.gpsimd.dma_start(ho_t[:, psl, :], ph[:])

                    # PE transposes -> 1-bank PSUM tiles -> DVE 2x copies
                    xT = work.tile([128, 2, GROUP, 128], F16, tag="xT")
                    hT = work.tile([128, 2, GROUP, 128], F16, tag="hT")
                    for a16, aT in ((x16, xT), (h16, hT)):
                        for c in range(2):
                            pt = ppx.tile([128, GROUP, 128], F16, tag="pt")
                            for j in range(GROUP):
                                nc.tensor.transpose(
                                    pt[:, j, :],
                                    a16[:, j, c * 128:(c + 1) * 128],
                                    ident[:])
                            nc.vector.tensor_copy(aT[:, c], pt[:])

                    gates = work.tile([128, GROUP, 1024], F16, tag="gates")
                    cg = work.tile([128, GROUP, 256], F16, tag="cg")
                    kp_o = io.tile([128, GROUP, H], F16, tag="kp_o")
                    k_o = io.tile([128, GROUP, H], F16, tag="k_o")
                    h_o = io.tile([128, GROUP, H], F16, tag="h_o")
                    HG = GROUP // 2
                    for half in range(2):
                        for j in range(half * HG, (half + 1) * HG):
                            ps = pp.tile([128, DG], F32, tag="ps")
                            # bias pre-fill: fp8 DoubleRow ones-matmuls
                            # (0.5 cyc/col), one per PSUM bank
                            for n0 in range(0, DG, 512):
                                n1 = min(n0 + 512, DG)
                                nc.tensor.matmul(ps[:, n0:n1], ones8[:],
                                                 b8[:, :, n0:n1],
                                                 start=True, stop=False,
                                                 perf_mode=DR)
                            for si, (side, aT) in enumerate((("x", xT),
                                                             ("h", hT))):
                                for c in range(2):
                                    lhsT = aT[:, c, j, :]
                                    last = si == 1 and c == 1
                                    for n0 in range(0, DG, 512):
                                        n1 = min(n0 + 512, DG)
                                        nc.tensor.matmul(
                                            ps[:, n0:n1], lhsT,
                                            WT[side, c][:, n0:n1],
                                            start=False, stop=last,
                                            skip_group_check=True)
                            nc.scalar.activation(gates[:, j, :],
                                                 ps[:, 0:1024], AF.Sigmoid)
                            nc.scalar.activation(cg[:, j, :],
                                                 ps[:, 1024:DG], AF.Tanh)

                        # fp16 elementwise tail for this half-group (DVE 2x)
                        hsl = slice(half * HG, (half + 1) * HG)
                        f_ = gates[:, hsl, 0:256]
                        i_ = gates[:, hsl, 256:512]
                        o_ = gates[:, hsl, 512:768]
                        u_ = gates[:, hsl, 768:1024]
                        kpp_h = kppr16[:, hsl, :]
                        d = tmp.tile([128, HG, H], F16, tag="d")
                        nc.vector.tensor_sub(d[:], cg[:, hsl, :], kpp_h)
                        e = tmp.tile([128, HG, H], F16, tag="e")
                        nc.vector.tensor_mul(e[:], u_, d[:])
                        m = tmp.tile([128, HG, H], F16, tag="m")
                        nc.vector.tensor_mul(m[:], f_, kpr16[:, hsl, :])
                        nc.vector.tensor_add(kp_o[:, hsl, :], e[:], kpp_h)
                        n2 = tmp.tile([128, HG, H], F16, tag="n2")
                        nc.vector.tensor_mul(n2[:], i_, kp_o[:, hsl, :])
                        nc.vector.tensor_add(k_o[:, hsl, :], m[:], n2[:])
                        tk = tmp.tile([128, HG, H], F16, tag="tk")
                        nc.scalar.activation(tk[:], k_o[:, hsl, :], AF.Tanh)
                        nc.vector.tensor_mul(h_o[:, hsl, :], o_, tk[:])

                        if gi == NG - 1:
                            # last group: store each half as soon as ready
                            gsl = slice(gi * GROUP + half * HG,
                                        gi * GROUP + (half + 1) * HG)
                            nc.gpsimd.dma_start(kpo_t[:, gsl, :],
                                                kp_o[:, hsl, :])
                            nc.gpsimd.dma_start(ko_t[:, gsl, :],
                                                k_o[:, hsl, :])
                            nc.gpsimd.dma_start(ho_t[:, gsl, :],
                                                h_o[:, hsl, :])

                    prev = (kp_o, k_o, h_o, nsl) if gi < NG - 1 else None

    nc.compile()
    _CACHE["nc"] = nc
    return nc


def kernel(x, h_prev, k_prev, kp_prev, Wx, bx, Uh, bh):
    x = np.asarray(x, dtype=np.float32)
    h_prev = np.asarray(h_prev, dtype=np.float32)
    k_prev = np.asarray(k_prev, dtype=np.float32)
    kp_prev = np.asarray(kp_prev, dtype=np.float32)
    Wx = np.ascontiguousarray(np.asarray(Wx, dtype=np.float32))
    bx = np.ascontiguousarray(np.asarray(bx, dtype=np.float32))
    Uh = np.ascontiguousarray(np.asarray(Uh, dtype=np.float32))
    bh = np.ascontiguousarray(np.asarray(bh, dtype=np.float32))

    nc = _build()
    in_maps = []
    for c in range(N_CORES):
        sl = slice(c * BL, (c + 1) * BL)
        in_maps.append({
            "x": np.ascontiguousarray(x[sl]),
            "h_prev": np.ascontiguousarray(h_prev[sl]),
            "k_prev": np.ascontiguousarray(k_prev[sl]),
            "kp_prev": np.ascontiguousarray(kp_prev[sl]),
            "Wx": Wx, "bx": bx, "Uh": Uh, "bh": bh,
        })
    res = run_bass_kernel_spmd(nc, in_maps, list(range(N_CORES)))
    h_out = np.concatenate([res.results[c]["h_out"] for c in range(N_CORES)],
                           axis=0)
    k_out = np.concatenate([res.results[c]["k_out"] for c in range(N_CORES)],
                           axis=0)
    kp_out = np.concatenate([res.results[c]["kp_out"]
                             for c in range(N_CORES)], axis=0)
    return (h_out, k_out, kp_out)
